# revision 14
# baseline (speedup 1.0000x reference)
import sys

sys.path.insert(0, "/opt/trn_rl_repo")

import numpy as np

P = 128          # partitions / tile edge
D = 128          # model dim
H = 4            # heads
DH = 32          # head dim
NCORES = 8

# Full-problem geometry (N=100000, E=800000). Each core owns NBLK node
# blocks of 128 nodes; every block's incident-edge list is padded to
# TBLK tiles of 128 edges so the SPMD program is uniform across cores.
NBLK_FULL = 98                      # 98*128 = 12544 own nodes/core
NPAD_FULL = NCORES * NBLK_FULL * P  # 100352 padded nodes


def _channel_perm():
    # torch reshape (N, DH, H): flat channel c = d*H + h. We relayout to
    # h-major c' = h*DH + d by permuting weight rows: perm[c'] = d*H + h.
    cp = np.arange(D)
    return (cp % DH) * H + (cp // DH)


def _build_program(NUSED_T, NOWN, NBLK, TBLK, capt, rows_bt):
    """Interleaved program: kv projection (in first-use node order) streams
    ahead while per-block edge gathers chase it.  Block b's gathers read only
    kv_d[0 : capt[b]*128], so they depend on a prefix of the projection
    stream and the Q7 gather pipeline (the critical resource) starts almost
    immediately instead of after the full projection."""
    import concourse.bass as bass
    import concourse.tile as tile
    from concourse import bacc, mybir
    from concourse.masks import make_identity
    from contextlib import ExitStack

    dt = mybir.dt
    f32, f16, bf16, i32 = dt.float32, dt.float16, dt.bfloat16, dt.int32
    NT = NBLK * TBLK      # edge tiles per core
    QT = NOWN // P        # x tiles for q projection (own nodes) == NBLK

    nc = bacc.Bacc("TRN2", target_bir_lowering=False, debug=False,
                   num_devices=NCORES)

    # x ships host-transposed (channel-major) so the contraction dim is
    # already on partitions: no PE transpose needed anywhere.  xt holds only
    # the nodes actually referenced by this core's edges, in first-use order.
    xt_d = nc.dram_tensor("xt", [D, NUSED_T * P], f16, kind="ExternalInput").ap()
    xot_d = nc.dram_tensor("xot", [D, NOWN], f16, kind="ExternalInput").ap()
    wkv_d = nc.dram_tensor("wkv", [D, 2 * D], f16, kind="ExternalInput").ap()
    wq_d = nc.dram_tensor("wq", [D, D], f16, kind="ExternalInput").ap()
    wo_d = nc.dram_tensor("wo", [D, D], f16, kind="ExternalInput").ap()
    bq_d = nc.dram_tensor("bq", [1, D], f16, kind="ExternalInput").ap()
    bo_d = nc.dram_tensor("bo", [1, D], f16, kind="ExternalInput").ap()
    ci_d = nc.dram_tensor("ci", [P, NT], i32, kind="ExternalInput").ap()
    selt_d = nc.dram_tensor("selt", [NBLK, P, TBLK * P], f16,
                            kind="ExternalInput").ap()
    rl_d = nc.dram_tensor("rl", [P, NT], f16, kind="ExternalInput").ap()
    io_d = nc.dram_tensor("io", [P, P], f16, kind="ExternalInput").ap()

    out_d = nc.dram_tensor("out", [NOWN, D], f32, kind="ExternalOutput").ap()
    kv_d = nc.dram_tensor("kv", [NUSED_T * P, 2 * D], f16).ap()
    q_d = nc.dram_tensor("q", [NOWN, D], f16).ap()

    AF = mybir.ActivationFunctionType
    OP = mybir.AluOpType

    with tile.TileContext(nc) as tc, ExitStack() as ctx:
        res = ctx.enter_context(tc.tile_pool(name="res", bufs=1))
        wkv_sb = res.tile([D, 2 * D], f16, name="wkv_sb")
        wq_sb = res.tile([D, D], f16, name="wq_sb")
        wo_sb = res.tile([D, D], f16, name="wo_sb")
        bq_sb = res.tile([1, D], f16, name="bq_sb")
        bo_sb = res.tile([1, D], f16, name="bo_sb")
        ci_sb = res.tile([P, NT], i32, name="ci_sb")

        rl_sb = res.tile([P, NT], f16, name="rl_sb")
        io_sb = res.tile([P, P], f16, name="io_sb")
        ones_sb = res.tile([1, P], f16, name="ones_sb")
        ident = res.tile([P, P], f16, name="ident")

        for sb_t, dr_t in [(wkv_sb, wkv_d), (wq_sb, wq_d), (wo_sb, wo_d),
                           (bq_sb, bq_d), (bo_sb, bo_d), (ci_sb, ci_d),
                           (rl_sb, rl_d), (io_sb, io_d)]:
            nc.sync.dma_start(sb_t[:], dr_t[:])
        nc.vector.memset(ones_sb[:], 1.0)
        make_identity(nc, ident[:])

        xa = ctx.enter_context(tc.tile_pool(name="xa", bufs=3))
        pa = ctx.enter_context(tc.tile_pool(name="pa", bufs=2, space="PSUM"))
        eg = ctx.enter_context(tc.tile_pool(name="eg", bufs=4))
        KGB = 12
        kg = ctx.enter_context(tc.tile_pool(name="kg", bufs=KGB))
        qx = ctx.enter_context(tc.tile_pool(name="qx", bufs=2, space="PSUM"))
        ep = ctx.enter_context(tc.tile_pool(name="ep", bufs=1, space="PSUM"))
        yp = ctx.enter_context(tc.tile_pool(name="yp", bufs=2, space="PSUM"))

        CH = 4

        def emit_q_tiles(j0, j1):
            # project own-node q tiles [j0, j1)
            for i in range(j0, j1, CH):
                c = min(CH, j1 - i)
                xo16 = xa.tile([P, c * P], f16, name="xo16")
                nc.sync.dma_start(xo16[:], xot_d[:, i * P:(i + c) * P])
                q4 = xa.tile([P, c, D], f16, name="q4")
                for t in range(c):
                    q_ps = pa.tile([P, 2 * D], f32, name="kv_ps")
                    nc.tensor.matmul(q_ps[:, 0:D], lhsT=ones_sb[:],
                                     rhs=bq_sb[:], start=True, stop=False)
                    nc.tensor.matmul(q_ps[:, 0:D],
                                     lhsT=xo16[:, t * P:(t + 1) * P],
                                     rhs=wq_sb[:], start=False, stop=True)
                    nc.scalar.copy(q4[:, t, :], q_ps[:, 0:D])
                nc.scalar.dma_start(
                    q_d[i * P:(i + c) * P, :].rearrange(
                        "(t p) c -> p t c", p=P), q4[:])

        # ---- interleaved kv projection + per-block gather/attention ----
        def emit_kv_tiles(i0, i1):
            # project x tiles [i0, i1) into kv rows [i0*P, i1*P)
            for j0 in range(i0, i1, CH):
                c = min(CH, i1 - j0)
                x16 = xa.tile([P, c * P], f16, name="x16")
                nc.sync.dma_start(x16[:], xt_d[:, j0 * P:(j0 + c) * P])
                kv4 = xa.tile([P, c, 2 * D], f16, name="kv4")
                for t in range(c):
                    kv_ps = pa.tile([P, 2 * D], f32, name="kv_ps")
                    nc.tensor.matmul(kv_ps[:],
                                     lhsT=x16[:, t * P:(t + 1) * P],
                                     rhs=wkv_sb[:], start=True, stop=True)
                    # split PSUM evacuation across the two free engines
                    if t % 2 == 0:
                        nc.vector.tensor_copy(kv4[:, t, :], kv_ps[:])
                    else:
                        nc.scalar.copy(kv4[:, t, :], kv_ps[:])
                nc.scalar.dma_start(
                    kv_d[j0 * P:(j0 + c) * P, :].rearrange(
                        "(t p) c -> p t c", p=P), kv4[:])

        LA = 10  # blocks of projection lookahead ahead of the gather stream
        written = 0
        qdone = 0
        for b in range(NBLK):
            tb = min(b + LA, NBLK - 1)
            if tb + 1 > qdone:
                emit_q_tiles(qdone, tb + 1)
                qdone = tb + 1
            if capt[tb] > written:
                emit_kv_tiles(written, capt[tb])
                written = capt[tb]
            T0 = b * TBLK
            kv_g = kg.tile([P, TBLK, 2 * D], f16, name="kv_g")
            kv_src = kv_d[0:capt[b] * P, :]
            for t in range(TBLK):
                # pad slots of the last tile keep stale-but-finite data from
                # the buffer's previous block; sel zeroes their contribution.
                # first KGB blocks gather fully to avoid uninitialized SBUF.
                r = P if b < KGB else int(rows_bt[b][t])
                if r == 0:
                    continue
                nc.gpsimd.indirect_dma_start(
                    out=kv_g[0:r, t, :], out_offset=None, in_=kv_src,
                    in_offset=bass.IndirectOffsetOnAxis(
                        ap=ci_sb[0:r, T0 + t:T0 + t + 1], axis=0))
            selt_b = eg.tile([P, TBLK * P], f16, name="selt_b")
            nc.sync.dma_start(selt_b[:], selt_d[b, :, :])
            qb = eg.tile([P, D], f16, name="qb")
            nc.sync.dma_start(qb[:], q_d[b * P:(b + 1) * P, :])

            sel = eg.tile([P, TBLK, P], bf16, name="sel")
            nc.vector.tensor_tensor(
                out=sel[:],
                in0=rl_sb[:, T0:T0 + TBLK].to_broadcast((P, TBLK, P)),
                in1=io_sb[:][:, None, :].to_broadcast((P, TBLK, P)),
                op=OP.is_equal)
            prod = eg.tile([P, TBLK, D], f16, name="prod")
            for t in range(TBLK):
                qx_ps = qx.tile([P, D], f32, name="qx_ps")
                nc.tensor.matmul(qx_ps[:],
                                 lhsT=selt_b[:, t * P:(t + 1) * P],
                                 rhs=qb[:], start=True, stop=True)
                nc.vector.tensor_tensor(out=prod[:, t, :], in0=qx_ps[:],
                                        in1=kv_g[:, t, 0:D], op=OP.mult)
            s_b = eg.tile([P, TBLK, H], f32, name="s_b")
            nc.vector.tensor_reduce(
                out=s_b[:],
                in_=prod[:].rearrange("p t (h d) -> p t h d", h=H),
                axis=mybir.AxisListType.X, op=OP.add)
            wext = eg.tile([P, TBLK, D + H], bf16, name="wext")
            nc.scalar.activation(wext[:, :, D:D + H], s_b[:], AF.Exp)
            nc.vector.tensor_tensor(
                out=wext[:, :, 0:D].rearrange("p t (h d) -> p t h d", h=H),
                in0=kv_g[:, :, D:2 * D].rearrange(
                    "p t (h d) -> p t h d", h=H),
                in1=wext[:, :, D:D + H].to_broadcast((P, TBLK, H, DH)),
                op=OP.mult)

            ypre = yp.tile([P, D + H], f32, name="ypre")
            for t in range(TBLK):
                nc.tensor.matmul(ypre[:], lhsT=sel[:, t, :],
                                 rhs=wext[:, t, :],
                                 start=(t == 0), stop=(t == TBLK - 1))

            zr = eg.tile([P, H], f32, name="zr")
            nc.vector.tensor_scalar_add(zr[:], ypre[:, D:D + H], 1e-30)
            rz = eg.tile([P, H], f32, name="rz")
            nc.vector.reciprocal(rz[:], zr[:])
            yb = eg.tile([P, D], f16, name="yb")
            nc.vector.tensor_tensor(
                out=yb[:].rearrange("p (h d) -> p h d", h=H),
                in0=ypre[:, 0:D].rearrange("p (h d) -> p h d", h=H),
                in1=rz[:].to_broadcast((P, H, DH)),
                op=OP.mult)
            yT_ps = ep.tile([P, D], f16, name="yT_ps")
            nc.tensor.transpose(yT_ps[:], yb[:], ident[:])
            yT = eg.tile([P, D], f16, name="yT")
            nc.scalar.copy(yT[:], yT_ps[:])
            o_ps = ep.tile([P, D], f32, name="o_ps")
            nc.tensor.matmul(o_ps[:], lhsT=ones_sb[:], rhs=bo_sb[:],
                             start=True, stop=False)
            nc.tensor.matmul(o_ps[:], lhsT=yT[:], rhs=wo_sb[:],
                             start=False, stop=True)
            o_sb = eg.tile([P, D], f32, name="o_sb")
            nc.scalar.copy(o_sb[:], o_ps[:])
            nc.scalar.dma_start(out_d[b * P:(b + 1) * P, :], o_sb[:])

        if NUSED_T > written:
            emit_kv_tiles(written, NUSED_T)

    nc.compile()
    return nc


def _prepare_inputs(x, row, col, Wq, bq, Wk, bk, Wv, bv, Wo, bo,
                    NUSED_T, NOWN, NBLK, TBLK):
    """Host-side sharding: per-core padded edge lists, first-use node
    ordering for the kv table, and permuted weights."""
    N = x.shape[0]
    perm = _channel_perm()
    s = np.sqrt(float(H))
    wkv_in = np.ascontiguousarray(
        np.concatenate([Wk[perm, :].T, Wv[perm, :].T], axis=1)
    ).astype(np.float16)
    wq_in = np.ascontiguousarray((Wq[perm, :] / s).T).astype(np.float16)
    wo_in = np.ascontiguousarray(Wo[:, perm].T).astype(np.float16)
    bq_in = (bq[perm] / s).reshape(1, D).astype(np.float16)
    # bv folds through the output projection exactly: sum_e a_e = 1.
    bo_in = (bo + Wo @ bv).reshape(1, D).astype(np.float16)
    io_in = np.tile(np.arange(P, dtype=np.float16), (P, 1))

    NPAD = NCORES * NOWN
    x_pad = np.zeros((NPAD, D), np.float32)
    x_pad[:N] = x

    NT = NBLK * TBLK
    EPC = NT * P  # padded edges per core
    in_maps = []
    for c in range(NCORES):
        lo, hi = c * NOWN, (c + 1) * NOWN
        e0 = np.searchsorted(row, lo, "left")
        e1 = np.searchsorted(row, hi, "left")
        rows_c = (row[e0:e1] - lo).astype(np.int64)
        cols_c = col[e0:e1].astype(np.int64)
        blk = rows_c // P
        blk_starts = np.searchsorted(blk, np.arange(NBLK), "left")
        rank = np.arange(rows_c.shape[0]) - blk_starts[blk]
        cnts = np.bincount(blk, minlength=NBLK)
        if cnts.max() > TBLK * P:
            raise ValueError(f"TBLK={TBLK} too small: need "
                             f"{int(np.ceil(cnts.max() / P))}")
        # first-use ordering of referenced nodes (edge stream is already
        # block-major because rows are sorted)
        uniq, first_idx = np.unique(cols_c, return_index=True)
        order = uniq[np.argsort(first_idx)]
        pos = np.zeros(NPAD, np.int64)
        pos[order] = np.arange(order.shape[0])
        xt_c = np.zeros((NUSED_T * P, D), np.float32)
        xt_c[:order.shape[0]] = x_pad[order]

        pos_e = pos[cols_c]
        posp = np.arange(rows_c.shape[0]) - blk_starts[blk] \
            + blk * (TBLK * P)
        ci = np.zeros(EPC, np.int32)
        rl = np.full(EPC, -1.0, np.float16)
        ci[posp] = pos_e.astype(np.int32)
        rl[posp] = (rows_c % P).astype(np.float16)
        selt = np.zeros((NBLK, P, TBLK * P), np.float16)
        selt[blk, rows_c % P, rank] = 1.0
        in_maps.append({
            "xt": np.ascontiguousarray(xt_c.T).astype(np.float16),
            "xot": np.ascontiguousarray(x_pad[lo:hi].T).astype(np.float16),
            "wkv": wkv_in, "wq": wq_in, "wo": wo_in,
            "bq": bq_in, "bo": bo_in,
            "ci": np.ascontiguousarray(ci.reshape(NT, P).T),
            "rl": np.ascontiguousarray(rl.reshape(NT, P).T),
            "io": io_in, "selt": selt,
        })
    return in_maps


def _edge_geometry(row, col, NOWN, NBLK):
    """TBLK, per-block gather caps (in 128-row tiles, max over cores), and
    the used-node tile count (uniform across cores)."""
    row = np.asarray(row, np.int64)
    col = np.asarray(col, np.int64)
    tblk = 1
    capt = np.zeros(NBLK, np.int64)
    nused_t = 1
    maxcnt = np.zeros(NBLK, np.int64)
    for c in range(NCORES):
        lo, hi = c * NOWN, (c + 1) * NOWN
        e0 = np.searchsorted(row, lo, "left")
        e1 = np.searchsorted(row, hi, "left")
        rows_c = row[e0:e1] - lo
        cols_c = col[e0:e1]
        blk = rows_c // P
        cnts = np.bincount(blk, minlength=NBLK)
        tblk = max(tblk, int(np.ceil(cnts.max() / P)))
        uniq, first_idx = np.unique(cols_c, return_index=True)
        nused_t = max(nused_t, int(np.ceil(uniq.shape[0] / P)))
        fs = np.sort(first_idx)
        ends = np.cumsum(cnts)  # edge index ends per block
        cum_distinct = np.searchsorted(fs, ends, "left")
        capt_c = np.ceil(cum_distinct / P).astype(np.int64)
        capt = np.maximum(capt, capt_c)
        maxcnt = np.maximum(maxcnt, cnts)
    capt = np.maximum(capt, 1)
    rows_bt = np.clip(maxcnt[:, None] - np.arange(tblk)[None, :] * P, 0, P)
    return tblk, capt, nused_t, rows_bt


def _install_ntff_hook():
    """The agent image's antenv lacks axon_hooks; inject it so trace=True
    can drive NTFF profiling through libaxon_pjrt.so."""
    import importlib
    try:
        importlib.import_module("antenv.axon_hooks")
        return
    except ImportError:
        pass
    import types
    if "/root/.axon_site" not in sys.path:
        sys.path.insert(0, "/root/.axon_site")
    from trn_agent_boot.trn_boot import _ntff_profile_via_ctypes
    hook = _ntff_profile_via_ctypes("/opt/axon/libaxon_pjrt.so")
    mod = types.ModuleType("antenv.axon_hooks")
    state = {"hook": hook}
    mod.get_axon_ntff_profile_hook = lambda: state["hook"]
    mod.set_axon_ntff_profile_hook = lambda h: state.update(hook=h)
    import antenv
    antenv.axon_hooks = mod
    sys.modules["antenv.axon_hooks"] = mod


def run(x, row, col, Wq, bq, Wk, bk, Wv, bv, Wo, bo, NBLK=NBLK_FULL,
        trace=False, tmpdir=None):
    from concourse import bass_utils
    from concourse.bass_utils import run_bass_kernel_spmd
    if trace:
        _install_ntff_hook()
        bass_utils.upload_artifacts = lambda d: "local://" + d

    x = np.asarray(x, np.float32)
    row = np.asarray(row, np.int64)
    col = np.asarray(col, np.int64)
    N = x.shape[0]
    NOWN = NBLK * P
    NPAD = NCORES * NOWN
    assert NPAD >= N
    TBLK, capt, NUSED_T, rows_bt = _edge_geometry(row, col, NOWN, NBLK)
    nc = _build_program(NUSED_T, NOWN, NBLK, TBLK, capt, rows_bt)
    in_maps = _prepare_inputs(
        x, row, col,
        np.asarray(Wq, np.float32), np.asarray(bq, np.float32),
        np.asarray(Wk, np.float32), np.asarray(bk, np.float32),
        np.asarray(Wv, np.float32), np.asarray(bv, np.float32),
        np.asarray(Wo, np.float32), np.asarray(bo, np.float32),
        NUSED_T, NOWN, NBLK, TBLK)
    res = run_bass_kernel_spmd(nc, in_maps, list(range(NCORES)), trace=trace,
                               tmpdir=tmpdir)
    out = np.concatenate([res.results[c]["out"] for c in range(NCORES)], 0)
    return out[:N].astype(np.float32), res


def kernel(**inputs):
    out, _ = run(**inputs)
    return out


# revision 17
# speedup vs baseline: 1.3695x; 1.3695x over previous
import sys

sys.path.insert(0, "/opt/trn_rl_repo")

import numpy as np

P = 128          # partitions / tile edge
D = 128          # model dim
H = 4            # heads
DH = 32          # head dim
NCORES = 8
GRP = 28         # destination blocks per kv-table group (window <= 32768 rows)

# Full-problem geometry (N=100000, E=800000). Each core owns NBLK node
# blocks of 128 nodes; every block's incident-edge list is padded to
# TBLK tiles of 128 edges so the SPMD program is uniform across cores.
NBLK_FULL = 98                      # 98*128 = 12544 own nodes/core
NPAD_FULL = NCORES * NBLK_FULL * P  # 100352 padded nodes


def _channel_perm():
    # torch reshape (N, DH, H): flat channel c = d*H + h. We relayout to
    # h-major c' = h*DH + d by permuting weight rows: perm[c'] = d*H + h.
    cp = np.arange(D)
    return (cp % DH) * H + (cp // DH)


def _build_program(NUSED_T, NOWN, NBLK, TBLK, capt, base_t, nia):
    """Interleaved program.  The kv table is grouped: blocks g*GRP..(g+1)*GRP
    share a <=32768-row window holding kv for exactly the nodes they
    reference (host-duplicated across groups), so each block's gather is ONE
    dma_gather with int16 window-local indices.  capt[b] (global tiles) caps
    the projection prefix each gather depends on; base_t[b] is the block's
    window base (tiles); nia[b] is its active index count."""
    import concourse.tile as tile
    from concourse import bacc, mybir
    from concourse.masks import make_identity
    from concourse.library_config import mlp
    from contextlib import ExitStack

    dt = mybir.dt
    f32, f16, bf16, i16 = dt.float32, dt.float16, dt.bfloat16, dt.int16
    NT = NBLK * TBLK      # edge tiles per core
    QT = NOWN // P        # x tiles for q projection (own nodes) == NBLK

    nc = bacc.Bacc("TRN2", target_bir_lowering=False, debug=False,
                   num_devices=NCORES)

    # x ships host-transposed (channel-major) so the contraction dim is
    # already on partitions.  Columns are in group-window order.
    xt_d = nc.dram_tensor("xt", [D, NUSED_T * P], f16, kind="ExternalInput").ap()
    xot_d = nc.dram_tensor("xot", [D, NOWN], f16, kind="ExternalInput").ap()
    wkv_d = nc.dram_tensor("wkv", [D, 2 * D], f16, kind="ExternalInput").ap()
    wq_d = nc.dram_tensor("wq", [D, D], f16, kind="ExternalInput").ap()
    wo_d = nc.dram_tensor("wo", [D, D], f16, kind="ExternalInput").ap()
    bq_d = nc.dram_tensor("bq", [1, D], f16, kind="ExternalInput").ap()
    bo_d = nc.dram_tensor("bo", [1, D], f16, kind="ExternalInput").ap()
    ix_d = nc.dram_tensor("ix", [P, NT * 8], i16, kind="ExternalInput").ap()
    selt_d = nc.dram_tensor("selt", [NBLK, P, TBLK * P], f16,
                            kind="ExternalInput").ap()
    rl_d = nc.dram_tensor("rl", [P, NT], f16, kind="ExternalInput").ap()
    io_d = nc.dram_tensor("io", [P, P], f16, kind="ExternalInput").ap()

    out_d = nc.dram_tensor("out", [NOWN, D], f32, kind="ExternalOutput").ap()
    kv_d = nc.dram_tensor("kv", [NUSED_T * P, 2 * D], f16).ap()
    q_d = nc.dram_tensor("q", [NOWN, D], f16).ap()

    AF = mybir.ActivationFunctionType
    OP = mybir.AluOpType

    with tile.TileContext(nc) as tc, ExitStack() as ctx:
        res = ctx.enter_context(tc.tile_pool(name="res", bufs=1))
        wkv_sb = res.tile([D, 2 * D], f16, name="wkv_sb")
        wq_sb = res.tile([D, D], f16, name="wq_sb")
        wo_sb = res.tile([D, D], f16, name="wo_sb")
        bq_sb = res.tile([1, D], f16, name="bq_sb")
        bo_sb = res.tile([1, D], f16, name="bo_sb")
        ix_sb = res.tile([P, NT * 8], i16, name="ix_sb")

        rl_sb = res.tile([P, NT], f16, name="rl_sb")
        io_sb = res.tile([P, P], f16, name="io_sb")
        ones_sb = res.tile([1, P], f16, name="ones_sb")
        ident = res.tile([P, P], f16, name="ident")

        for sb_t, dr_t in [(wkv_sb, wkv_d), (wq_sb, wq_d), (wo_sb, wo_d),
                           (bq_sb, bq_d), (bo_sb, bo_d), (ix_sb, ix_d),
                           (rl_sb, rl_d), (io_sb, io_d)]:
            nc.sync.dma_start(sb_t[:], dr_t[:])
        nc.vector.memset(ones_sb[:], 1.0)
        make_identity(nc, ident[:])
        nc.gpsimd.load_library(mlp)

        xa = ctx.enter_context(tc.tile_pool(name="xa", bufs=3))
        pa = ctx.enter_context(tc.tile_pool(name="pa", bufs=2, space="PSUM"))
        eg = ctx.enter_context(tc.tile_pool(name="eg", bufs=4))
        KGB = 8
        kg = ctx.enter_context(tc.tile_pool(name="kg", bufs=KGB))
        qx = ctx.enter_context(tc.tile_pool(name="qx", bufs=2, space="PSUM"))
        ep = ctx.enter_context(tc.tile_pool(name="ep", bufs=1, space="PSUM"))
        yp = ctx.enter_context(tc.tile_pool(name="yp", bufs=2, space="PSUM"))

        CH = 4

        def emit_q_tiles(j0, j1):
            # project own-node q tiles [j0, j1)
            for i in range(j0, j1, CH):
                c = min(CH, j1 - i)
                xo16 = xa.tile([P, c * P], f16, name="xo16")
                nc.sync.dma_start(xo16[:], xot_d[:, i * P:(i + c) * P])
                q4 = xa.tile([P, c, D], f16, name="q4")
                for t in range(c):
                    q_ps = pa.tile([P, 2 * D], f32, name="kv_ps")
                    nc.tensor.matmul(q_ps[:, 0:D], lhsT=ones_sb[:],
                                     rhs=bq_sb[:], start=True, stop=False)
                    nc.tensor.matmul(q_ps[:, 0:D],
                                     lhsT=xo16[:, t * P:(t + 1) * P],
                                     rhs=wq_sb[:], start=False, stop=True)
                    nc.scalar.copy(q4[:, t, :], q_ps[:, 0:D])
                nc.scalar.dma_start(
                    q_d[i * P:(i + c) * P, :].rearrange(
                        "(t p) c -> p t c", p=P), q4[:])

        def emit_kv_tiles(i0, i1):
            # project x tiles [i0, i1) into kv rows [i0*P, i1*P)
            for j0 in range(i0, i1, CH):
                c = min(CH, i1 - j0)
                x16 = xa.tile([P, c * P], f16, name="x16")
                nc.sync.dma_start(x16[:], xt_d[:, j0 * P:(j0 + c) * P])
                kv4 = xa.tile([P, c, 2 * D], f16, name="kv4")
                for t in range(c):
                    kv_ps = pa.tile([P, 2 * D], f32, name="kv_ps")
                    nc.tensor.matmul(kv_ps[:],
                                     lhsT=x16[:, t * P:(t + 1) * P],
                                     rhs=wkv_sb[:], start=True, stop=True)
                    # split PSUM evacuation across the two free engines
                    if t % 2 == 0:
                        nc.vector.tensor_copy(kv4[:, t, :], kv_ps[:])
                    else:
                        nc.scalar.copy(kv4[:, t, :], kv_ps[:])
                nc.scalar.dma_start(
                    kv_d[j0 * P:(j0 + c) * P, :].rearrange(
                        "(t p) c -> p t c", p=P), kv4[:])

        LA = 6  # blocks of projection lookahead ahead of the gather stream
        written = 0
        qdone = 0
        for b in range(NBLK):
            tb = min(b + LA, NBLK - 1)
            if tb + 1 > qdone:
                emit_q_tiles(qdone, tb + 1)
                qdone = tb + 1
            if capt[tb] > written:
                emit_kv_tiles(written, capt[tb])
                written = capt[tb]
            T0 = b * TBLK
            kv_g = kg.tile([P, TBLK, 2 * D], f16, name="kv_g")
            # one gather for the whole block; indices are window-local.
            # first KGB blocks gather all padded slots so no SBUF stays
            # uninitialized; afterwards the stale tail (sel==0) is harmless.
            ni = TBLK * P if b < KGB else int(nia[b])
            nt = (ni + P - 1) // P
            nc.gpsimd.dma_gather(
                kv_g[:, 0:nt, :],
                kv_d[base_t[b] * P:capt[b] * P, :],
                ix_sb[:, T0 * 8:(T0 + nt) * 8],
                ni, ni, 2 * D, single_packet=False)
            selt_b = eg.tile([P, TBLK * P], f16, name="selt_b")
            nc.sync.dma_start(selt_b[:], selt_d[b, :, :])
            qb = eg.tile([P, D], f16, name="qb")
            nc.sync.dma_start(qb[:], q_d[b * P:(b + 1) * P, :])

            sel = eg.tile([P, TBLK, P], bf16, name="sel")
            nc.vector.tensor_tensor(
                out=sel[:],
                in0=rl_sb[:, T0:T0 + TBLK].to_broadcast((P, TBLK, P)),
                in1=io_sb[:][:, None, :].to_broadcast((P, TBLK, P)),
                op=OP.is_equal)
            prod = eg.tile([P, TBLK, D], f16, name="prod")
            for t in range(TBLK):
                qx_ps = qx.tile([P, D], f32, name="qx_ps")
                nc.tensor.matmul(qx_ps[:],
                                 lhsT=selt_b[:, t * P:(t + 1) * P],
                                 rhs=qb[:], start=True, stop=True)
                nc.vector.tensor_tensor(out=prod[:, t, :], in0=qx_ps[:],
                                        in1=kv_g[:, t, 0:D], op=OP.mult)
            s_b = eg.tile([P, TBLK, H], f32, name="s_b")
            nc.vector.tensor_reduce(
                out=s_b[:],
                in_=prod[:].rearrange("p t (h d) -> p t h d", h=H),
                axis=mybir.AxisListType.X, op=OP.add)
            wext = eg.tile([P, TBLK, D + H], bf16, name="wext")
            nc.scalar.activation(wext[:, :, D:D + H], s_b[:], AF.Exp)
            nc.vector.tensor_tensor(
                out=wext[:, :, 0:D].rearrange("p t (h d) -> p t h d", h=H),
                in0=kv_g[:, :, D:2 * D].rearrange(
                    "p t (h d) -> p t h d", h=H),
                in1=wext[:, :, D:D + H].to_broadcast((P, TBLK, H, DH)),
                op=OP.mult)

            ypre = yp.tile([P, D + H], f32, name="ypre")
            for t in range(TBLK):
                nc.tensor.matmul(ypre[:], lhsT=sel[:, t, :],
                                 rhs=wext[:, t, :],
                                 start=(t == 0), stop=(t == TBLK - 1))

            zr = eg.tile([P, H], f32, name="zr")
            nc.vector.tensor_scalar_add(zr[:], ypre[:, D:D + H], 1e-30)
            rz = eg.tile([P, H], f32, name="rz")
            nc.vector.reciprocal(rz[:], zr[:])
            yb = eg.tile([P, D], f16, name="yb")
            nc.vector.tensor_tensor(
                out=yb[:].rearrange("p (h d) -> p h d", h=H),
                in0=ypre[:, 0:D].rearrange("p (h d) -> p h d", h=H),
                in1=rz[:].to_broadcast((P, H, DH)),
                op=OP.mult)
            yT_ps = ep.tile([P, D], f16, name="yT_ps")
            nc.tensor.transpose(yT_ps[:], yb[:], ident[:])
            yT = eg.tile([P, D], f16, name="yT")
            nc.scalar.copy(yT[:], yT_ps[:])
            o_ps = ep.tile([P, D], f32, name="o_ps")
            nc.tensor.matmul(o_ps[:], lhsT=ones_sb[:], rhs=bo_sb[:],
                             start=True, stop=False)
            nc.tensor.matmul(o_ps[:], lhsT=yT[:], rhs=wo_sb[:],
                             start=False, stop=True)
            o_sb = eg.tile([P, D], f32, name="o_sb")
            nc.scalar.copy(o_sb[:], o_ps[:])
            nc.scalar.dma_start(out_d[b * P:(b + 1) * P, :], o_sb[:])

        if NUSED_T > written:
            emit_kv_tiles(written, NUSED_T)

    nc.compile()
    return nc


def _group_geometry(row, col, NOWN, NBLK):
    """Per-core group windows (first-use order within each block group) and
    the uniform (max-over-core) shapes: TBLK, window sizes, per-block caps,
    active index counts."""
    row = np.asarray(row, np.int64)
    col = np.asarray(col, np.int64)
    NG = (NBLK + GRP - 1) // GRP
    tblk = 1
    maxcnt = np.zeros(NBLK, np.int64)
    wtiles = np.zeros(NG, np.int64)          # window size per group (tiles)
    capg = np.zeros(NBLK, np.int64)          # within-group cap per block
    percore = []
    for c in range(NCORES):
        lo, hi = c * NOWN, (c + 1) * NOWN
        e0 = np.searchsorted(row, lo, "left")
        e1 = np.searchsorted(row, hi, "left")
        rows_c = row[e0:e1] - lo
        cols_c = col[e0:e1]
        blk = rows_c // P
        cnts = np.bincount(blk, minlength=NBLK)
        ends = np.cumsum(cnts)
        tblk = max(tblk, int(np.ceil(cnts.max() / P)))
        maxcnt = np.maximum(maxcnt, cnts)
        orders = []
        wpos_e = np.zeros(cols_c.shape[0], np.int64)
        for g in range(NG):
            b0, b1 = g * GRP, min((g + 1) * GRP, NBLK)
            ge0 = 0 if b0 == 0 else ends[b0 - 1]
            ge1 = ends[b1 - 1]
            gc = cols_c[ge0:ge1]
            uniq, first_idx = np.unique(gc, return_index=True)
            order = uniq[np.argsort(first_idx)]
            pos = np.zeros(NCORES * NOWN, np.int64)
            pos[order] = np.arange(order.shape[0])
            wpos_e[ge0:ge1] = pos[gc]
            assert order.shape[0] <= 32768
            wtiles[g] = max(wtiles[g], int(np.ceil(order.shape[0] / P)))
            fs = np.sort(first_idx)
            gends = ends[b0:b1] - ge0
            cum = np.searchsorted(fs, gends, "left")
            capg[b0:b1] = np.maximum(
                capg[b0:b1], np.ceil(cum / P).astype(np.int64))
            orders.append(order)
        percore.append({"orders": orders, "wpos_e": wpos_e,
                        "e0": e0, "e1": e1})
    capg = np.maximum(capg, 1)
    base_t = np.zeros(NBLK, np.int64)
    off = 0
    for g in range(NG):
        b0, b1 = g * GRP, min((g + 1) * GRP, NBLK)
        base_t[b0:b1] = off
        off += wtiles[g]
    capt = base_t + capg          # global projection cap per block (tiles)
    nused_t = int(off)
    nia = maxcnt                  # active (non-pad) gather idxs per block
    return tblk, int(nused_t), base_t, capt, nia, wtiles, percore


def _prepare_inputs(x, row, col, Wq, bq, Wk, bk, Wv, bv, Wo, bo,
                    geo, NOWN, NBLK, TBLK):
    """Host-side sharding: per-core padded edge lists, group-window node
    duplication for the kv table, int16 gather indices, permuted weights."""
    tblk, NUSED_T, base_t, capt, nia, wtiles, percore = geo
    N = x.shape[0]
    perm = _channel_perm()
    s = np.sqrt(float(H))
    wkv_in = np.ascontiguousarray(
        np.concatenate([Wk[perm, :].T, Wv[perm, :].T], axis=1)
    ).astype(np.float16)
    wq_in = np.ascontiguousarray((Wq[perm, :] / s).T).astype(np.float16)
    wo_in = np.ascontiguousarray(Wo[:, perm].T).astype(np.float16)
    bq_in = (bq[perm] / s).reshape(1, D).astype(np.float16)
    # bv folds through the output projection exactly: sum_e a_e = 1.
    bo_in = (bo + Wo @ bv).reshape(1, D).astype(np.float16)
    io_in = np.tile(np.arange(P, dtype=np.float16), (P, 1))

    NPAD = NCORES * NOWN
    x_pad = np.zeros((NPAD, D), np.float32)
    x_pad[:N] = x

    NG = (NBLK + GRP - 1) // GRP
    NT = NBLK * TBLK
    EPC = NT * P  # padded edges per core
    in_maps = []
    for c in range(NCORES):
        pc = percore[c]
        e0, e1 = pc["e0"], pc["e1"]
        rows_c = (row[e0:e1] - c * NOWN).astype(np.int64)
        blk = rows_c // P
        blk_starts = np.searchsorted(blk, np.arange(NBLK), "left")
        rank = np.arange(rows_c.shape[0]) - blk_starts[blk]

        # kv-table input: per-group first-use node order, zero-padded windows
        xt_c = np.zeros((NUSED_T * P, D), np.float32)
        for g in range(NG):
            order = pc["orders"][g]
            o0 = base_t[g * GRP] * P
            xt_c[o0:o0 + order.shape[0]] = x_pad[order]

        posp = rank + blk * (TBLK * P)
        ci = np.zeros(EPC, np.int16)
        rl = np.full(EPC, -1.0, np.float16)
        ci[posp] = pc["wpos_e"].astype(np.int16)
        rl[posp] = (rows_c % P).astype(np.float16)
        # wrapped int16 index layout: flat slot j of block b lives at
        # partition j%16 (replicated across the 8 groups of 16), free slot
        # b*(TBLK*P//16) + j//16
        ciw = ci.reshape(NBLK, TBLK * P)
        ixw = np.zeros((NBLK, 16, TBLK * P // 16), np.int16)
        j = np.arange(TBLK * P)
        ixw[:, j % 16, j // 16] = ciw[:, j]
        ix = np.tile(ixw.transpose(1, 0, 2).reshape(16, NT * 8), (8, 1))

        selt = np.zeros((NBLK, P, TBLK * P), np.float16)
        selt[blk, rows_c % P, rank] = 1.0
        in_maps.append({
            "xt": np.ascontiguousarray(xt_c.T).astype(np.float16),
            "xot": np.ascontiguousarray(x_pad[c * NOWN:(c + 1) * NOWN].T
                                        ).astype(np.float16),
            "wkv": wkv_in, "wq": wq_in, "wo": wo_in,
            "bq": bq_in, "bo": bo_in,
            "ix": np.ascontiguousarray(ix),
            "rl": np.ascontiguousarray(rl.reshape(NT, P).T),
            "io": io_in, "selt": selt,
        })
    return in_maps


def _install_ntff_hook():
    """The agent image's antenv lacks axon_hooks; inject it so trace=True
    can drive NTFF profiling through libaxon_pjrt.so."""
    import importlib
    try:
        importlib.import_module("antenv.axon_hooks")
        return
    except ImportError:
        pass
    import types
    if "/root/.axon_site" not in sys.path:
        sys.path.insert(0, "/root/.axon_site")
    from trn_agent_boot.trn_boot import _ntff_profile_via_ctypes
    hook = _ntff_profile_via_ctypes("/opt/axon/libaxon_pjrt.so")
    mod = types.ModuleType("antenv.axon_hooks")
    state = {"hook": hook}
    mod.get_axon_ntff_profile_hook = lambda: state["hook"]
    mod.set_axon_ntff_profile_hook = lambda h: state.update(hook=h)
    import antenv
    antenv.axon_hooks = mod
    sys.modules["antenv.axon_hooks"] = mod


def run(x, row, col, Wq, bq, Wk, bk, Wv, bv, Wo, bo, NBLK=NBLK_FULL,
        trace=False, tmpdir=None):
    from concourse import bass_utils
    from concourse.bass_utils import run_bass_kernel_spmd
    if trace:
        _install_ntff_hook()
        bass_utils.upload_artifacts = lambda d: "local://" + d

    x = np.asarray(x, np.float32)
    row = np.asarray(row, np.int64)
    col = np.asarray(col, np.int64)
    N = x.shape[0]
    NOWN = NBLK * P
    NPAD = NCORES * NOWN
    assert NPAD >= N
    geo = _group_geometry(row, col, NOWN, NBLK)
    TBLK, NUSED_T, base_t, capt, nia = geo[0], geo[1], geo[2], geo[3], geo[4]
    nc = _build_program(NUSED_T, NOWN, NBLK, TBLK, capt, base_t, nia)
    in_maps = _prepare_inputs(
        x, row, col,
        np.asarray(Wq, np.float32), np.asarray(bq, np.float32),
        np.asarray(Wk, np.float32), np.asarray(bk, np.float32),
        np.asarray(Wv, np.float32), np.asarray(bv, np.float32),
        np.asarray(Wo, np.float32), np.asarray(bo, np.float32),
        geo, NOWN, NBLK, TBLK)
    res = run_bass_kernel_spmd(nc, in_maps, list(range(NCORES)), trace=trace,
                               tmpdir=tmpdir)
    out = np.concatenate([res.results[c]["out"] for c in range(NCORES)], 0)
    return out[:N].astype(np.float32), res


def kernel(**inputs):
    out, _ = run(**inputs)
    return out


# revision 23
# speedup vs baseline: 1.3696x; 1.0001x over previous
import sys

sys.path.insert(0, "/opt/trn_rl_repo")

import numpy as np

P = 128          # partitions / tile edge
D = 128          # model dim
H = 4            # heads
DH = 32          # head dim
NCORES = 8
GRP = 28         # destination blocks per kv-table group (window <= 32768 rows)
KGB = 8          # kv_g pool depth; first KGB blocks gather fully (init SBUF)

# Full-problem geometry (N=100000, E=800000). Each core owns NBLK node
# blocks of 128 nodes; every block's incident-edge list is padded to
# TBLK tiles of 128 edges so the SPMD program is uniform across cores.
NBLK_FULL = 98                      # 98*128 = 12544 own nodes/core
NPAD_FULL = NCORES * NBLK_FULL * P  # 100352 padded nodes


def _channel_perm():
    # torch reshape (N, DH, H): flat channel c = d*H + h. We relayout to
    # h-major c' = h*DH + d by permuting weight rows: perm[c'] = d*H + h.
    cp = np.arange(D)
    return (cp % DH) * H + (cp // DH)


def _build_program(NUSED_T, NOWN, NBLK, TBLK, capt, base_t, nia):
    """Interleaved program.  The kv table is grouped: blocks g*GRP..(g+1)*GRP
    share a <=32768-row window holding kv for exactly the nodes they
    reference (host-duplicated across groups), so each block's gather is ONE
    dma_gather with int16 window-local indices.  capt[b] (global tiles) caps
    the projection prefix each gather depends on; base_t[b] is the block's
    window base (tiles); nia[b] is its active index count."""
    import concourse.tile as tile
    from concourse import bacc, mybir
    from concourse.masks import make_identity
    from concourse.library_config import mlp
    from contextlib import ExitStack

    dt = mybir.dt
    f32, f16, bf16, i16 = dt.float32, dt.float16, dt.bfloat16, dt.int16
    NT = NBLK * TBLK      # edge tiles per core
    QT = NOWN // P        # x tiles for q projection (own nodes) == NBLK

    nc = bacc.Bacc("TRN2", target_bir_lowering=False, debug=False,
                   num_devices=NCORES)

    # x ships host-transposed (channel-major) so the contraction dim is
    # already on partitions.  Columns are in group-window order.
    xt_d = nc.dram_tensor("xt", [D, NUSED_T * P], f16, kind="ExternalInput").ap()
    xot_d = nc.dram_tensor("xot", [D, NOWN], f16, kind="ExternalInput").ap()
    wkv_d = nc.dram_tensor("wkv", [D, 2 * D], f16, kind="ExternalInput").ap()
    wq_d = nc.dram_tensor("wq", [D, D], f16, kind="ExternalInput").ap()
    wo_d = nc.dram_tensor("wo", [D, D], f16, kind="ExternalInput").ap()
    bq_d = nc.dram_tensor("bq", [1, D], f16, kind="ExternalInput").ap()
    bo_d = nc.dram_tensor("bo", [1, D], f16, kind="ExternalInput").ap()
    ix_d = nc.dram_tensor("ix", [P, NT * 8], i16, kind="ExternalInput").ap()
    selt_d = nc.dram_tensor("selt", [NBLK, P, TBLK * P], f16,
                            kind="ExternalInput").ap()
    rl_d = nc.dram_tensor("rl", [P, NT], f16, kind="ExternalInput").ap()
    io_d = nc.dram_tensor("io", [P, P], f16, kind="ExternalInput").ap()

    out_d = nc.dram_tensor("out", [NOWN, D], f32, kind="ExternalOutput").ap()
    kv_d = nc.dram_tensor("kv", [NUSED_T * P, 2 * D], f16).ap()
    q_d = nc.dram_tensor("q", [NOWN, D], f16).ap()

    AF = mybir.ActivationFunctionType
    OP = mybir.AluOpType

    with tile.TileContext(nc) as tc, ExitStack() as ctx:
        res = ctx.enter_context(tc.tile_pool(name="res", bufs=1))
        wkv_sb = res.tile([D, 2 * D], f16, name="wkv_sb")
        wq_sb = res.tile([D, D], f16, name="wq_sb")
        wo_sb = res.tile([D, D], f16, name="wo_sb")
        bq_sb = res.tile([1, D], f16, name="bq_sb")
        bo_sb = res.tile([1, D], f16, name="bo_sb")
        ix_sb = res.tile([P, NT * 8], i16, name="ix_sb")

        rl_sb = res.tile([P, NT], f16, name="rl_sb")
        io_sb = res.tile([P, P], f16, name="io_sb")
        ones_sb = res.tile([1, P], f16, name="ones_sb")
        ident = res.tile([P, P], f16, name="ident")

        for sb_t, dr_t in [(wkv_sb, wkv_d), (wq_sb, wq_d), (wo_sb, wo_d),
                           (bq_sb, bq_d), (bo_sb, bo_d), (ix_sb, ix_d),
                           (rl_sb, rl_d), (io_sb, io_d)]:
            nc.sync.dma_start(sb_t[:], dr_t[:])
        nc.vector.memset(ones_sb[:], 1.0)
        make_identity(nc, ident[:])
        nc.gpsimd.load_library(mlp)

        xa = ctx.enter_context(tc.tile_pool(name="xa", bufs=3))
        pa = ctx.enter_context(tc.tile_pool(name="pa", bufs=2, space="PSUM"))
        eg = ctx.enter_context(tc.tile_pool(name="eg", bufs=4))
        kg = ctx.enter_context(tc.tile_pool(name="kg", bufs=KGB))
        qx = ctx.enter_context(tc.tile_pool(name="qx", bufs=1, space="PSUM"))
        ep = ctx.enter_context(tc.tile_pool(name="ep", bufs=1, space="PSUM"))
        yp = ctx.enter_context(tc.tile_pool(name="yp", bufs=1, space="PSUM"))

        CH = 4

        def emit_q_tiles(j0, j1):
            # project own-node q tiles [j0, j1)
            for i in range(j0, j1, CH):
                c = min(CH, j1 - i)
                xo16 = xa.tile([P, c * P], f16, name="xo16")
                nc.sync.dma_start(xo16[:], xot_d[:, i * P:(i + c) * P])
                q4 = xa.tile([P, c, D], f16, name="q4")
                for t in range(c):
                    q_ps = pa.tile([P, 2 * D], f32, name="kv_ps")
                    nc.tensor.matmul(q_ps[:, 0:D], lhsT=ones_sb[:],
                                     rhs=bq_sb[:], start=True, stop=False)
                    nc.tensor.matmul(q_ps[:, 0:D],
                                     lhsT=xo16[:, t * P:(t + 1) * P],
                                     rhs=wq_sb[:], start=False, stop=True)
                    nc.scalar.copy(q4[:, t, :], q_ps[:, 0:D])
                nc.scalar.dma_start(
                    q_d[i * P:(i + c) * P, :].rearrange(
                        "(t p) c -> p t c", p=P), q4[:])

        def emit_kv_tiles(i0, i1):
            # project x tiles [i0, i1) into kv rows [i0*P, i1*P)
            for j0 in range(i0, i1, CH):
                c = min(CH, i1 - j0)
                x16 = xa.tile([P, c * P], f16, name="x16")
                nc.sync.dma_start(x16[:], xt_d[:, j0 * P:(j0 + c) * P])
                kv4 = xa.tile([P, c, 2 * D], f16, name="kv4")
                for t in range(c):
                    kv_ps = pa.tile([P, 2 * D], f32, name="kv_ps")
                    nc.tensor.matmul(kv_ps[:],
                                     lhsT=x16[:, t * P:(t + 1) * P],
                                     rhs=wkv_sb[:], start=True, stop=True)
                    # split PSUM evacuation across the two free engines
                    if t % 2 == 0:
                        nc.vector.tensor_copy(kv4[:, t, :], kv_ps[:])
                    else:
                        nc.scalar.copy(kv4[:, t, :], kv_ps[:])
                nc.scalar.dma_start(
                    kv_d[j0 * P:(j0 + c) * P, :].rearrange(
                        "(t p) c -> p t c", p=P), kv4[:])

        LA = 6  # blocks of projection lookahead ahead of the gather stream
        written = 0
        qdone = 0
        for b in range(NBLK):
            tb = min(b + LA, NBLK - 1)
            if tb + 1 > qdone:
                emit_q_tiles(qdone, tb + 1)
                qdone = tb + 1
            if capt[tb] > written:
                emit_kv_tiles(written, capt[tb])
                written = capt[tb]
            T0 = b * TBLK
            kv_g = kg.tile([P, TBLK, 2 * D], f16, name="kv_g")
            # one gather for the whole block; indices are window-local.
            # first KGB blocks gather all padded slots so no SBUF stays
            # uninitialized; afterwards the stale tail (sel==0) is harmless.
            ni = TBLK * P if b < KGB else int(nia[b])
            nt = (ni + P - 1) // P
            nc.gpsimd.dma_gather(
                kv_g[:, 0:nt, :],
                kv_d[base_t[b] * P:capt[b] * P, :],
                ix_sb[:, T0 * 8:(T0 + nt) * 8],
                ni, ni, 2 * D, single_packet=False)
            selt_b = eg.tile([P, TBLK * P], f16, name="selt_b")
            nc.sync.dma_start(selt_b[:], selt_d[b, :, :])
            qb = eg.tile([P, D], f16, name="qb")
            nc.sync.dma_start(qb[:], q_d[b * P:(b + 1) * P, :])

            sel = eg.tile([P, TBLK, P], bf16, name="sel")
            nc.vector.tensor_tensor(
                out=sel[:],
                in0=rl_sb[:, T0:T0 + TBLK].to_broadcast((P, TBLK, P)),
                in1=io_sb[:][:, None, :].to_broadcast((P, TBLK, P)),
                op=OP.is_equal)
            prod = eg.tile([P, TBLK, D], f16, name="prod")
            qx_ps = qx.tile([P, TBLK, D], f32, name="qx_ps")
            for t in range(TBLK):
                nc.tensor.matmul(qx_ps[:, t, :],
                                 lhsT=selt_b[:, t * P:(t + 1) * P],
                                 rhs=qb[:], start=True, stop=True)
            nc.vector.tensor_tensor(out=prod[:], in0=qx_ps[:],
                                    in1=kv_g[:, :, 0:D], op=OP.mult)
            s_b = eg.tile([P, TBLK, H], f32, name="s_b")
            nc.vector.tensor_reduce(
                out=s_b[:].rearrange("p t h -> p (t h)"),
                in_=prod[:].rearrange("p t (h d) -> p (t h) d", h=H),
                axis=mybir.AxisListType.X, op=OP.add)
            wext = eg.tile([P, TBLK, D + H], bf16, name="wext")
            nc.scalar.activation(wext[:, :, D:D + H], s_b[:], AF.Exp)
            nc.vector.tensor_tensor(
                out=wext[:, :, 0:D].rearrange("p t (h d) -> p t h d", h=H),
                in0=kv_g[:, :, D:2 * D].rearrange(
                    "p t (h d) -> p t h d", h=H),
                in1=wext[:, :, D:D + H].to_broadcast((P, TBLK, H, DH)),
                op=OP.mult)

            ypre = yp.tile([P, D + H], f32, name="ypre")
            for t in range(TBLK):
                nc.tensor.matmul(ypre[:], lhsT=sel[:, t, :],
                                 rhs=wext[:, t, :],
                                 start=(t == 0), stop=(t == TBLK - 1))

            zr = eg.tile([P, H], f32, name="zr")
            nc.vector.tensor_scalar_add(zr[:], ypre[:, D:D + H], 1e-30)
            rz = eg.tile([P, H], f32, name="rz")
            nc.vector.reciprocal(rz[:], zr[:])
            yb = eg.tile([P, D], f16, name="yb")
            nc.vector.tensor_tensor(
                out=yb[:].rearrange("p (h d) -> p h d", h=H),
                in0=ypre[:, 0:D].rearrange("p (h d) -> p h d", h=H),
                in1=rz[:].to_broadcast((P, H, DH)),
                op=OP.mult)
            yT_ps = ep.tile([P, D], f16, name="yT_ps")
            nc.tensor.transpose(yT_ps[:], yb[:], ident[:])
            yT = eg.tile([P, D], f16, name="yT")
            nc.scalar.copy(yT[:], yT_ps[:])
            o_ps = ep.tile([P, D], f32, name="o_ps")
            nc.tensor.matmul(o_ps[:], lhsT=ones_sb[:], rhs=bo_sb[:],
                             start=True, stop=False)
            nc.tensor.matmul(o_ps[:], lhsT=yT[:], rhs=wo_sb[:],
                             start=False, stop=True)
            o_sb = eg.tile([P, D], f32, name="o_sb")
            nc.scalar.copy(o_sb[:], o_ps[:])
            nc.scalar.dma_start(out_d[b * P:(b + 1) * P, :], o_sb[:])

        if NUSED_T > written:
            emit_kv_tiles(written, NUSED_T)

    nc.compile()
    return nc


def _group_geometry(row, col, NOWN, NBLK):
    """Per-core group windows (first-use order within each block group) and
    the uniform (max-over-core) shapes: TBLK, window sizes, per-block caps,
    active index counts."""
    row = np.asarray(row, np.int64)
    col = np.asarray(col, np.int64)
    NG = (NBLK + GRP - 1) // GRP
    tblk = 1
    maxcnt = np.zeros(NBLK, np.int64)
    wtiles = np.zeros(NG, np.int64)          # window size per group (tiles)
    capg = np.zeros(NBLK, np.int64)          # within-group cap per block
    percore = []
    for c in range(NCORES):
        lo, hi = c * NOWN, (c + 1) * NOWN
        e0 = np.searchsorted(row, lo, "left")
        e1 = np.searchsorted(row, hi, "left")
        rows_c = row[e0:e1] - lo
        cols_c = col[e0:e1]
        blk = rows_c // P
        cnts = np.bincount(blk, minlength=NBLK)
        ends = np.cumsum(cnts)
        tblk = max(tblk, int(np.ceil(cnts.max() / P)))
        maxcnt = np.maximum(maxcnt, cnts)
        orders = []
        wpos_e = np.zeros(cols_c.shape[0], np.int64)
        for g in range(NG):
            b0, b1 = g * GRP, min((g + 1) * GRP, NBLK)
            ge0 = 0 if b0 == 0 else ends[b0 - 1]
            ge1 = ends[b1 - 1]
            gc = cols_c[ge0:ge1]
            uniq, first_idx = np.unique(gc, return_index=True)
            order = uniq[np.argsort(first_idx)]
            pos = np.zeros(NCORES * NOWN, np.int64)
            pos[order] = np.arange(order.shape[0])
            wpos_e[ge0:ge1] = pos[gc]
            assert order.shape[0] <= 32768
            wtiles[g] = max(wtiles[g], int(np.ceil(order.shape[0] / P)))
            fs = np.sort(first_idx)
            gends = ends[b0:b1] - ge0
            cum = np.searchsorted(fs, gends, "left")
            capg[b0:b1] = np.maximum(
                capg[b0:b1], np.ceil(cum / P).astype(np.int64))
            orders.append(order)
        percore.append({"orders": orders, "wpos_e": wpos_e,
                        "e0": e0, "e1": e1})
    capg = np.maximum(capg, 1)
    base_t = np.zeros(NBLK, np.int64)
    off = 0
    for g in range(NG):
        b0, b1 = g * GRP, min((g + 1) * GRP, NBLK)
        base_t[b0:b1] = off
        off += wtiles[g]
    capt = base_t + capg          # global projection cap per block (tiles)
    nused_t = int(off)
    nia = maxcnt                  # active (non-pad) gather idxs per block
    return tblk, int(nused_t), base_t, capt, nia, wtiles, percore


def _prepare_inputs(x, row, col, Wq, bq, Wk, bk, Wv, bv, Wo, bo,
                    geo, NOWN, NBLK, TBLK):
    """Host-side sharding: per-core padded edge lists, group-window node
    duplication for the kv table, int16 gather indices, permuted weights."""
    tblk, NUSED_T, base_t, capt, nia, wtiles, percore = geo
    N = x.shape[0]
    perm = _channel_perm()
    s = np.sqrt(float(H))
    wkv_in = np.ascontiguousarray(
        np.concatenate([Wk[perm, :].T, Wv[perm, :].T], axis=1)
    ).astype(np.float16)
    wq_in = np.ascontiguousarray((Wq[perm, :] / s).T).astype(np.float16)
    wo_in = np.ascontiguousarray(Wo[:, perm].T).astype(np.float16)
    bq_in = (bq[perm] / s).reshape(1, D).astype(np.float16)
    # bv folds through the output projection exactly: sum_e a_e = 1.
    bo_in = (bo + Wo @ bv).reshape(1, D).astype(np.float16)
    io_in = np.tile(np.arange(P, dtype=np.float16), (P, 1))

    NPAD = NCORES * NOWN
    x_pad = np.zeros((NPAD, D), np.float32)
    x_pad[:N] = x

    NG = (NBLK + GRP - 1) // GRP
    NT = NBLK * TBLK
    EPC = NT * P  # padded edges per core
    in_maps = []
    for c in range(NCORES):
        pc = percore[c]
        e0, e1 = pc["e0"], pc["e1"]
        rows_c = (row[e0:e1] - c * NOWN).astype(np.int64)
        blk = rows_c // P
        blk_starts = np.searchsorted(blk, np.arange(NBLK), "left")
        rank = np.arange(rows_c.shape[0]) - blk_starts[blk]

        # kv-table input: per-group first-use node order, zero-padded windows
        xt_c = np.zeros((NUSED_T * P, D), np.float32)
        for g in range(NG):
            order = pc["orders"][g]
            o0 = base_t[g * GRP] * P
            xt_c[o0:o0 + order.shape[0]] = x_pad[order]

        posp = rank + blk * (TBLK * P)
        ci = np.zeros(EPC, np.int16)
        rl = np.full(EPC, -1.0, np.float16)
        ci[posp] = pc["wpos_e"].astype(np.int16)
        rl[posp] = (rows_c % P).astype(np.float16)
        # wrapped int16 index layout: flat slot j of block b lives at
        # partition j%16 (replicated across the 8 groups of 16), free slot
        # b*(TBLK*P//16) + j//16
        ciw = ci.reshape(NBLK, TBLK * P)
        ixw = np.zeros((NBLK, 16, TBLK * P // 16), np.int16)
        j = np.arange(TBLK * P)
        ixw[:, j % 16, j // 16] = ciw[:, j]
        ix = np.tile(ixw.transpose(1, 0, 2).reshape(16, NT * 8), (8, 1))

        selt = np.zeros((NBLK, P, TBLK * P), np.float16)
        selt[blk, rows_c % P, rank] = 1.0
        in_maps.append({
            "xt": np.ascontiguousarray(xt_c.T).astype(np.float16),
            "xot": np.ascontiguousarray(x_pad[c * NOWN:(c + 1) * NOWN].T
                                        ).astype(np.float16),
            "wkv": wkv_in, "wq": wq_in, "wo": wo_in,
            "bq": bq_in, "bo": bo_in,
            "ix": np.ascontiguousarray(ix),
            "rl": np.ascontiguousarray(rl.reshape(NT, P).T),
            "io": io_in, "selt": selt,
        })
    return in_maps


def _install_ntff_hook():
    """The agent image's antenv lacks axon_hooks; inject it so trace=True
    can drive NTFF profiling through libaxon_pjrt.so."""
    import importlib
    try:
        importlib.import_module("antenv.axon_hooks")
        return
    except ImportError:
        pass
    import types
    if "/root/.axon_site" not in sys.path:
        sys.path.insert(0, "/root/.axon_site")
    from trn_agent_boot.trn_boot import _ntff_profile_via_ctypes
    hook = _ntff_profile_via_ctypes("/opt/axon/libaxon_pjrt.so")
    mod = types.ModuleType("antenv.axon_hooks")
    state = {"hook": hook}
    mod.get_axon_ntff_profile_hook = lambda: state["hook"]
    mod.set_axon_ntff_profile_hook = lambda h: state.update(hook=h)
    import antenv
    antenv.axon_hooks = mod
    sys.modules["antenv.axon_hooks"] = mod


def run(x, row, col, Wq, bq, Wk, bk, Wv, bv, Wo, bo, NBLK=NBLK_FULL,
        trace=False, tmpdir=None):
    from concourse import bass_utils
    from concourse.bass_utils import run_bass_kernel_spmd
    if trace:
        _install_ntff_hook()
        bass_utils.upload_artifacts = lambda d: "local://" + d

    x = np.asarray(x, np.float32)
    row = np.asarray(row, np.int64)
    col = np.asarray(col, np.int64)
    N = x.shape[0]
    NOWN = NBLK * P
    NPAD = NCORES * NOWN
    assert NPAD >= N
    geo = _group_geometry(row, col, NOWN, NBLK)
    TBLK, NUSED_T, base_t, capt, nia = geo[0], geo[1], geo[2], geo[3], geo[4]
    nc = _build_program(NUSED_T, NOWN, NBLK, TBLK, capt, base_t, nia)
    in_maps = _prepare_inputs(
        x, row, col,
        np.asarray(Wq, np.float32), np.asarray(bq, np.float32),
        np.asarray(Wk, np.float32), np.asarray(bk, np.float32),
        np.asarray(Wv, np.float32), np.asarray(bv, np.float32),
        np.asarray(Wo, np.float32), np.asarray(bo, np.float32),
        geo, NOWN, NBLK, TBLK)
    res = run_bass_kernel_spmd(nc, in_maps, list(range(NCORES)), trace=trace,
                               tmpdir=tmpdir)
    out = np.concatenate([res.results[c]["out"] for c in range(NCORES)], 0)
    return out[:N].astype(np.float32), res


def kernel(**inputs):
    out, _ = run(**inputs)
    return out
